# revision 1
# baseline (speedup 1.0000x reference)
"""Bass/Trainium2 kernel for nn_Attn (Bahdanau 'general' attention scoring).

Reference math:
    energies = einsum('sd,hd,h->s', enc, W, hidden) + b.hidden
    out      = softmax(energies)[None, None, :]

Factorization:
    v = W^T @ hidden (200-dim), energies = enc @ v (+ const; softmax cancels
    the constant b.hidden term, so b is dropped).

Distribution (8 NeuronCores, one TRN2 chip) — d-sharding:
  The per-execution collective-bootstrap on this runtime costs ~20-40us, so
  local work is arranged to hide under it and the wire traffic is minimal.
  - Core i owns d-slice [25*i, 25*(i+1)) of the contraction dim:
      W slice  [8192, 25]  -> v_i = W_i^T @ hidden (exact, local, no comm)
      enc slice [32768, 25] -> partial energies e_i[s] = enc[s, d_i] . v_i
    for ALL 32768 positions, laid out [128, 256] (s = p*256 + f).
  - ONE AllReduce(add) over the 128KB partials -> full energies everywhere.
  - Every core computes the identical softmax and writes the full output;
    the host takes core 0's copy.
  Partial energies are computed on the VectorEngine (contraction of 25 is
  PE-hostile); PE only does tiny transposes / partition reduce-broadcasts.
"""

import numpy as np

N_CORES = 8
SEQ = 32768
D = 200
H = 8192
DSH = D // N_CORES      # 25
P = 128
F = SEQ // P            # 256
KCH = H // P            # 64
NCH = 4                 # enc DMA / DVE chunks along F
FC = F // NCH           # 64
PSH = P // N_CORES      # 16 partition rows per core after ReduceScatter
S_LOCAL = PSH * F       # 4096 output positions per core


def build_kernel():
    import concourse.bacc as bacc
    import concourse.bass as bass
    import concourse.mybir as mybir
    import concourse.tile as tile
    from concourse import masks

    fp32 = mybir.dt.float32
    nc = bacc.Bacc(
        "TRN2",
        target_bir_lowering=False,
        debug=False,
        num_devices=N_CORES,
    )

    # Host-prepacked layouts (see shard_inputs):
    #   encP [128, 256*25]: [p, f, d] with global s = p*256 + f
    #   wP   [128, 25*64]:  [p, d, k] with h = k*128 + p  (d-major!)
    #   hidP [128, 64]:     [p, k]    with h = k*128 + p
    encP = nc.dram_tensor("encP", [P, F * DSH], fp32, kind="ExternalInput")
    wP = nc.dram_tensor("wP", [P, DSH * KCH], fp32, kind="ExternalInput")
    hidP = nc.dram_tensor("hidP", [P, KCH], fp32, kind="ExternalInput")
    out = nc.dram_tensor("out", [SEQ], fp32, kind="ExternalOutput")
    # Sink for the warm-up collective (kept live so it isn't DCE'd).
    warm_out = nc.dram_tensor("warm_out", [2, 4], fp32,
                              kind="ExternalOutput")

    rg = [list(range(N_CORES))]

    with tile.TileContext(nc) as tc:
        with (
            tc.tile_pool(name="const", bufs=1) as constp,
            tc.tile_pool(name="sb", bufs=1) as sb,
            tc.tile_pool(name="ps", bufs=1, space="PSUM") as ps,
            tc.tile_pool(name="dram", bufs=1, space="DRAM") as dram,
        ):
            # ---- warm-up collective, FIRST and with NO data dependencies:
            # a tiny AllGather over an unwritten scratch tile (payload is
            # irrelevant). The FIRST collective of an execution pays a fixed
            # ~11.3us setup after the runtime's entry barrier; this dummy
            # absorbs that cost while the DMAs/DVE work run, so the real
            # AllReduce below only pays the ~1.2us subsequent-collective
            # latency. Dependency-free so the Tile scheduler cannot sink
            # the trigger behind the DMA chain.
            warm_b = nc.inline_tensor(np.zeros((1, 4), np.float32),
                                      name="warm_src")
            # 2-core pair groups: same rendezvous-absorbing effect, but the
            # pairwise mesh completes faster than the 8-core one (~4us vs 8).
            warm_g = dram.tile([2, 4], fp32)
            nc.gpsimd.collective_compute(
                "AllGather",
                mybir.AluOpType.bypass,
                replica_groups=[[2 * i, 2 * i + 1] for i in range(N_CORES // 2)],
                ins=[warm_b.ap().opt()],
                outs=[warm_g[:].opt()],
            )
            # Keep it live. NOT on the gpsimd queue: its wait on the warm
            # AllGather would hold back the AllReduce trigger below (gpsimd
            # executes in order). Scalar is idle in this window.
            nc.scalar.dma_start(warm_out.ap(), warm_g[:])

            ones = constp.tile([128, 128], fp32)
            nc.vector.memset(ones[:], 1.0)
            ident = constp.tile([128, 128], fp32)
            masks.make_identity(nc, ident[:])

            # ---- loads (w + hid first: they gate the v chain) ----
            w_sb = sb.tile([P, DSH * KCH], fp32)
            nc.sync.dma_start(w_sb[:], wP.ap())
            h_sb = sb.tile([P, KCH], fp32)
            nc.sync.dma_start(h_sb[:], hidP.ap())

            enc_sb = sb.tile([P, F * DSH], fp32)
            for c in range(NCH):
                sl = slice(c * FC * DSH, (c + 1) * FC * DSH)
                nc.sync.dma_start(enc_sb[:, sl], encP.ap()[:, sl])

            # ---- v_i = W_i^T @ hidden (DVE mul + unit-stride reduce) ----
            prod_w = sb.tile([P, DSH * KCH], fp32)
            h_b = (
                h_sb[:]
                .rearrange("p k -> p () k")
                .broadcast_to([P, DSH, KCH])
            )
            nc.vector.tensor_tensor(
                out=prod_w[:].rearrange("p (d k) -> p d k", d=DSH),
                in0=w_sb[:].rearrange("p (d k) -> p d k", d=DSH),
                in1=h_b,
                op=mybir.AluOpType.mult,
            )
            vtmp = sb.tile([P, DSH], fp32)
            nc.vector.reduce_sum(
                vtmp[:],
                prod_w[:].rearrange("p (d k) -> p d k", d=DSH),
                axis=mybir.AxisListType.X,
            )
            # one matmul: column-sums broadcast to every partition
            v_bc_ps = ps.tile([P, DSH], fp32, tag="vbc")
            nc.tensor.matmul(
                v_bc_ps[:], lhsT=ones[:], rhs=vtmp[:], start=True, stop=True
            )
            v_bc = sb.tile([P, DSH], fp32)
            nc.scalar.copy(v_bc[:], v_bc_ps[:])

            # ---- partial energies e_i[p, f] = sum_d enc[p, f, d] * v[d] ----
            e_part = sb.tile([P, F], fp32)
            for c in range(NCH):
                sl3 = enc_sb[:].rearrange("p (f d) -> p f d", d=DSH)[
                    :, c * FC : (c + 1) * FC, :
                ]
                eprod = sb.tile([P, FC * DSH], fp32, tag="eprod", bufs=2)
                v_b = (
                    v_bc[:]
                    .rearrange("p d -> p () d")
                    .broadcast_to([P, FC, DSH])
                )
                nc.vector.tensor_tensor(
                    out=eprod[:].rearrange("p (f d) -> p f d", d=DSH),
                    in0=sl3,
                    in1=v_b,
                    op=mybir.AluOpType.mult,
                )
                nc.vector.reduce_sum(
                    e_part[:, c * FC : (c + 1) * FC],
                    eprod[:].rearrange("p (f d) -> p f d", d=DSH),
                    axis=mybir.AxisListType.X,
                )

            # ---- AllReduce the partial energies ----
            bounce = dram.tile([P, F], fp32)
            esum = dram.tile([P, F], fp32, addr_space="Shared")
            nc.sync.dma_start(bounce[:, 0 : F // 2], e_part[:, 0 : F // 2])
            nc.sync.dma_start(bounce[:, F // 2 : F], e_part[:, F // 2 : F])
            nc.gpsimd.collective_compute(
                "AllReduce",
                mybir.AluOpType.add,
                replica_groups=rg,
                ins=[bounce[:].opt()],
                outs=[esum[:].opt()],
            )
            e_sb = sb.tile([P, F], fp32)
            nc.sync.dma_start(e_sb[:], esum[:])

            # ---- replicated softmax over [128, 256] ----
            m_p = sb.tile([P, 1], fp32)
            nc.vector.reduce_max(m_p[:], e_sb[:], axis=mybir.AxisListType.X)
            mt = ps.tile([1, P], fp32, tag="mt")
            nc.tensor.transpose(mt[:], m_p[:], ident[:])
            negM = sb.tile([1, 1], fp32)
            nc.vector.reduce_max(
                negM[:], mt[:], axis=mybir.AxisListType.X, negate=True
            )
            negM_ps = ps.tile([P, 1], fp32, tag="negm")
            nc.tensor.matmul(
                negM_ps[:], lhsT=ones[0:1, :], rhs=negM[:], start=True, stop=True
            )
            negM_bc = sb.tile([P, 1], fp32)
            nc.scalar.copy(negM_bc[:], negM_ps[:])

            p_sb = sb.tile([P, F], fp32)
            s_p = sb.tile([P, 1], fp32)
            nc.scalar.activation(
                p_sb[:], e_sb[:],
                mybir.ActivationFunctionType.Exp,
                bias=negM_bc[:], scale=1.0,
                accum_out=s_p[:],
            )
            S_ps = ps.tile([P, 1], fp32, tag="S")
            nc.tensor.matmul(
                S_ps[:], lhsT=ones[:], rhs=s_p[:], start=True, stop=True
            )
            rS = sb.tile([P, 1], fp32)
            nc.vector.reciprocal(rS[:], S_ps[:])
            o_sb = sb.tile([P, F], fp32)
            nc.vector.tensor_scalar_mul(o_sb[:], p_sb[:], rS[:])
            nc.sync.dma_start(out.ap().rearrange("(p f) -> p f", p=P), o_sb[:])

    nc.compile()
    return nc


def shard_inputs(hidden, encoder_outputs, W, b):
    hidden = np.asarray(hidden, dtype=np.float32)
    enc = np.asarray(encoder_outputs, dtype=np.float32)
    W = np.asarray(W, dtype=np.float32)
    enc3 = enc.reshape(P, F, D)          # s = p*F + f
    w3 = W.reshape(KCH, P, D)            # h = k*P + p
    hidP = np.ascontiguousarray(hidden.reshape(KCH, P).T)  # [p, k]
    in_maps = []
    for i in range(N_CORES):
        sl = slice(i * DSH, (i + 1) * DSH)
        encP_i = np.ascontiguousarray(enc3[:, :, sl]).reshape(P, F * DSH)
        wP_i = np.ascontiguousarray(
            w3[:, :, sl].transpose(1, 2, 0)       # [p, d, k]
        ).reshape(P, DSH * KCH)
        in_maps.append({"encP": encP_i, "wP": wP_i, "hidP": hidP})
    return in_maps


_NC_CACHE = {}


def _get_nc():
    if "nc" not in _NC_CACHE:
        _NC_CACHE["nc"] = build_kernel()
    return _NC_CACHE["nc"]


def kernel(hidden, encoder_outputs, W, b):
    from concourse import bass_utils

    nc = _get_nc()
    in_maps = shard_inputs(hidden, encoder_outputs, W, b)
    res = bass_utils.run_bass_kernel_spmd(
        nc, in_maps, core_ids=list(range(N_CORES))
    )
    out = np.asarray(res.results[0]["out"], dtype=np.float32)
    return out.reshape(1, 1, SEQ)

